# revision 12
# baseline (speedup 1.0000x reference)
"""Trainium2 Bass kernel for nn_Decoder_86011015069969.

Pointer-network decoder: B=16 items, each runs M=64 sequential steps of
pointer attention over S=256 candidates with Gumbel-max sampling.

Strategy (pure data parallelism): 8 cores x 2 items/core, one SPMD program.
Host precomputes only input-independent PRNG constants (Gumbel noise from
fixed seed 42 — exactly what jax.random.categorical adds to the logits).

Device-side algebra (validated against the jax reference on host):
  - query recurrence folded:  a_i = q_i @ Wq = c1 + P[idx_{i-1}],
    P = emb @ Wv[256:] @ Wq,  Q = emb @ Wv[:256] @ Wq,  c1 = c0 + Q[idx_0]
  - sampling: argmax_j( tanh(u_j) + M10_j + g_ij/10 ), scale-invariant vs
    reference argmax(10*tanh(u) + mask*(-1e9) + g); M10 in {0, -1e8}
  - log_softmax via fixed-shift logsumexp: lse = 10 + log(sum exp(lg-10))
"""
import numpy as np
from contextlib import ExitStack

import concourse.bass as bass
import concourse.bacc as bacc
import concourse.tile as tile
import concourse.mybir as mybir
from concourse.bass_utils import run_bass_kernel_spmd

F32 = mybir.dt.float32
AF = mybir.ActivationFunctionType
OP = mybir.AluOpType

B, S, E, H, M = 16, 256, 256, 256, 64
NCORES = 8
JPC = B // NCORES  # items per core = 2
DEBUG = False


# ----------------------------------------------------------------------------
# device program
# ----------------------------------------------------------------------------

def build_nc():
    nc = bacc.Bacc("TRN2", target_bir_lowering=False, debug=False,
                   enable_asserts=True)

    # --- DRAM I/O (per core) ---
    emb_d = nc.dram_tensor("emb", [JPC, S, E], F32, kind="ExternalInput")
    ncst_d = nc.dram_tensor("ncst", [JPC, S, 5], F32, kind="ExternalInput")
    g10_d = nc.dram_tensor("g10", [JPC, M, S], F32, kind="ExternalInput")
    Wk_d = nc.dram_tensor("Wk", [E, H], F32, kind="ExternalInput")
    Wq_d = nc.dram_tensor("Wq", [E, H], F32, kind="ExternalInput")
    Whc_d = nc.dram_tensor("Whc", [E, E], F32, kind="ExternalInput")
    Wv_d = nc.dram_tensor("Wv", [2 * E, E], F32, kind="ExternalInput")
    bb_d = nc.dram_tensor("bb", [E], F32, kind="ExternalInput")      # bhc + bv
    vptr_d = nc.dram_tensor("vptr", [H], F32, kind="ExternalInput")
    iw_d = nc.dram_tensor("iw", [2 * E], F32, kind="ExternalInput")
    ident_d = nc.dram_tensor("ident", [128, 128], F32, kind="ExternalInput")
    cneg_d = nc.dram_tensor("cneg", [128, 128], F32, kind="ExternalInput")
    iota_d = nc.dram_tensor("iota", [128, 2], F32, kind="ExternalInput")

    lp_d = nc.dram_tensor("lp", [JPC], F32, kind="ExternalOutput")
    rw_d = nc.dram_tensor("rw", [JPC], F32, kind="ExternalOutput")
    act_d = nc.dram_tensor("act", [JPC, M], F32, kind="ExternalOutput")

    with tile.TileContext(nc) as tc, ExitStack() as ctx:
        sb = ctx.enter_context(tc.tile_pool(name="sb", bufs=1))
        tpool = ctx.enter_context(tc.tile_pool(name="tp", bufs=2))
        ps = ctx.enter_context(tc.tile_pool(name="ps", bufs=1, space="PSUM"))
        scr = ctx.enter_context(tc.tile_pool(name="scr", bufs=2, space="PSUM"))

        def mm(out, lhsT, rhs, start, stop):
            nc.tensor.matmul(out, lhsT, rhs, start=start, stop=stop,
                             skip_group_check=True)

        def st(shape, nm):
            return sb.tile(shape, F32, name=nm, tag=nm)

        def scrt():
            return scr.tile([128, 256], F32, name="scrA", tag="scrA")

        # --- SBUF constants / weights ---
        Wk_sb = st([128, 2 * H], "Wk_sb")
        Wq_sb = st([128, 2 * H], "Wq_sb")
        Whc_sb = st([128, 2 * E], "Whc_sb")
        Wv_sb = st([128, 4 * E], "Wv_sb")
        bb_row = st([1, E], "bb_row")
        vptr_col = st([128, 2], "vptr_col")
        iw_col = st([128, 4], "iw_col")
        ident = st([128, 128], "ident_sb")
        cneg = st([128, 128], "cneg_sb")
        iota_col = st([128, 2], "iota_sb")
        nc.sync.dma_start(Wk_sb[:].rearrange("p (r c) -> p r c", r=2),
                          Wk_d.ap().rearrange("(r p) c -> p r c", p=128))
        nc.sync.dma_start(Wq_sb[:].rearrange("p (r c) -> p r c", r=2),
                          Wq_d.ap().rearrange("(r p) c -> p r c", p=128))
        nc.sync.dma_start(Whc_sb[:].rearrange("p (r c) -> p r c", r=2),
                          Whc_d.ap().rearrange("(r p) c -> p r c", p=128))
        nc.sync.dma_start(Wv_sb[:].rearrange("p (r c) -> p r c", r=4),
                          Wv_d.ap().rearrange("(r p) c -> p r c", p=128))
        nc.sync.dma_start(bb_row[:], bb_d.ap().rearrange("(a c) -> a c", a=1))
        nc.sync.dma_start(vptr_col[:], vptr_d.ap().rearrange("(c p) -> p c", p=128))
        nc.sync.dma_start(iw_col[:], iw_d.ap().rearrange("(c p) -> p c", p=128))
        nc.sync.dma_start(ident[:], ident_d.ap())
        nc.sync.dma_start(cneg[:], cneg_d.ap())
        nc.sync.dma_start(iota_col[:], iota_d.ap())

        one11 = st([1, 1], "one11")
        ones_col = st([128, 1], "ones_col")
        inv_col = st([128, 1], "inv_col")
        zrow = st([1, 128], "zrow")
        negrow = st([1, M], "negrow")
        bm10 = st([128, 1], "bm10")
        nc.vector.memset(one11[:], 1.0)
        nc.vector.memset(ones_col[:], 1.0)
        nc.vector.memset(inv_col[:], 1.0 / S)
        nc.vector.memset(zrow[:], 0.0)
        nc.vector.memset(negrow[:], -1.0)
        nc.vector.memset(bm10[:], -10.0)

        # --- shared precompute: WvaT/WvbT -> Gq (= Wva@Wq), G (= Wvb@Wq) ---
        WvaT = st([128, 2 * E], "WvaT")
        WvbT = st([128, 2 * E], "WvbT")
        for half, dst in ((0, WvaT), (1, WvbT)):
            for cc in range(2):
                p = scrt()
                for rc in range(2):
                    nc.tensor.transpose(
                        p[:, rc * 128:(rc + 1) * 128],
                        Wv_sb[:, (2 * half + rc) * 256 + cc * 128:
                              (2 * half + rc) * 256 + (cc + 1) * 128],
                        ident[:])
                nc.scalar.copy(dst[:, cc * 256:(cc + 1) * 256], p[:])
        G_sb = st([128, 2 * H], "G_sb")
        Gq_sb = st([128, 2 * H], "Gq_sb")
        for src, dst in ((WvbT, G_sb), (WvaT, Gq_sb)):
            for vc in range(2):
                p = scrt()
                for mc in range(2):
                    mm(p[:], src[:, mc * 256 + vc * 128: mc * 256 + vc * 128 + 128],
                       Wq_sb[:, mc * 256:(mc + 1) * 256],
                       start=(mc == 0), stop=(mc == 1))
                nc.scalar.copy(dst[:, vc * 256:(vc + 1) * 256], p[:])

        # t1 = init_w @ Wv  (shared, col layout)
        t1_sb = st([128, 2], "t1_sb")
        p = scrt()
        for ec in range(2):
            for rc in range(4):
                mm(p[:, ec:ec + 1],
                   Wv_sb[:, rc * 256 + ec * 128: rc * 256 + ec * 128 + 128],
                   iw_col[:, rc:rc + 1], start=(rc == 0), stop=(rc == 3))
        nc.vector.tensor_copy(t1_sb[:], p[:, 0:2])

        # --- per-item tiles ---
        embN, embT, kT, P_sb, Q_sb, NC_sb, g10sb = [], [], [], [], [], [], []
        tum, oh, a_sb, c0row, c1row, tu_tmp = [], [], [], [], [], []
        z_row, oh_row, mx, meanv, hv_sb = [], [], [], [], []
        for j in range(JPC):
            embN.append(st([128, 2 * E], f"embN{j}"))
            embT.append(st([128, 2 * S], f"embT{j}"))
            kT.append(st([128, 2 * S], f"kT{j}"))
            P_sb.append(st([128, 2 * H], f"P{j}"))
            Q_sb.append(st([128, 2 * H], f"Q{j}"))
            NC_sb.append(st([128, 10], f"NC{j}"))
            g10sb.append(st([1, M, S], f"g10sb{j}"))
            tum.append(st([128, 2, 66], f"tum{j}"))
            oh.append(st([128, 2, 66], f"oh{j}"))
            a_sb.append(st([128, 2], f"a_sb{j}"))
            c0row.append(st([1, H], f"c0row{j}"))
            c1row.append(st([1, H], f"c1row{j}"))
            tu_tmp.append(st([128, 2], f"tu_tmp{j}"))
            z_row.append(st([1, S], f"z_row{j}"))
            oh_row.append(st([1, S], f"oh_row{j}"))
            mx.append(st([1, 1], f"mx{j}"))
            meanv.append(st([128, 2], f"meanv{j}"))
            hv_sb.append(st([128, 2], f"hv{j}"))

        ubank = [ps.tile([128, 512], F32, name=f"ub{j}", tag=f"ub{j}")
                 for j in range(JPC)]
        abank = [ps.tile([128, 512], F32, name=f"ab{j}", tag=f"ab{j}")
                 for j in range(JPC)]
        mbank = [ps.tile([128, 512], F32, name=f"mb{j}", tag=f"mb{j}")
                 for j in range(JPC)]

        for j in range(JPC):
            nc.sync.dma_start(embN[j][:].rearrange("p (r c) -> p r c", r=2),
                              emb_d.ap()[j].rearrange("(r p) c -> p r c", p=128))
            nc.sync.dma_start(NC_sb[j][:].rearrange("p (r c) -> p r c", r=2),
                              ncst_d.ap()[j].rearrange("(r p) c -> p r c", p=128))
            nc.sync.dma_start(g10sb[j][:],
                              g10_d.ap()[j].rearrange("(a m) s -> a m s", a=1))
            nc.vector.memset(tum[j][:], 0.0)
            nc.vector.memset(oh[j][:], 0.0)

        # --- per-item setup ---
        for j in range(JPC):
            # embT via PE transposes
            for ec in range(2):
                p = scrt()
                for sc in range(2):
                    nc.tensor.transpose(
                        p[:, sc * 128:(sc + 1) * 128],
                        embN[j][:, sc * 256 + ec * 128: sc * 256 + ec * 128 + 128],
                        ident[:])
                nc.scalar.copy(embT[j][:, ec * 256:(ec + 1) * 256], p[:])
            # kT[h,s] = sum_e Wk[e,h] * embT[e,s]
            for hc in range(2):
                p = scrt()
                for ec in range(2):
                    mm(p[:], Wk_sb[:, ec * 256 + hc * 128: ec * 256 + hc * 128 + 128],
                       embT[j][:, ec * 256:(ec + 1) * 256],
                       start=(ec == 0), stop=(ec == 1))
                nc.scalar.copy(kT[j][:, hc * 256:(hc + 1) * 256], p[:])
            # P = emb @ G ; Q = emb @ Gq   (natural [s, h] layout)
            for gsb, dst in ((G_sb, P_sb[j]), (Gq_sb, Q_sb[j])):
                for sc in range(2):
                    p = scrt()
                    for ec in range(2):
                        mm(p[:],
                           embT[j][:, ec * 256 + sc * 128: ec * 256 + sc * 128 + 128],
                           gsb[:, ec * 256:(ec + 1) * 256],
                           start=(ec == 0), stop=(ec == 1))
                    nc.scalar.copy(dst[:, sc * 256:(sc + 1) * 256], p[:])
            # meanv_col = mean_s emb
            p = scrt()
            for ec in range(2):
                for sc in range(2):
                    mm(p[:, ec:ec + 1],
                       embN[j][:, sc * 256 + ec * 128: sc * 256 + ec * 128 + 128],
                       inv_col[:], start=(sc == 0), stop=(sc == 1))
            nc.vector.tensor_copy(meanv[j][:], p[:, 0:2])
            # hv = meanv @ Whc + (bhc + bv)   (col layout)
            p = scrt()
            for oc in range(2):
                mm(p[:, oc:oc + 1], bb_row[0:1, oc * 128:(oc + 1) * 128], one11[:],
                   start=True, stop=False)
                for ec in range(2):
                    mm(p[:, oc:oc + 1],
                       Whc_sb[:, ec * 256 + oc * 128: ec * 256 + oc * 128 + 128],
                       meanv[j][:, ec:ec + 1], start=False, stop=(ec == 1))
            nc.vector.tensor_copy(hv_sb[j][:], p[:, 0:2])
            # c0_row = hv @ Wq
            p = scrt()
            for ec in range(2):
                mm(p[0:1, :], hv_sb[j][:, ec:ec + 1],
                   Wq_sb[:, ec * 256:(ec + 1) * 256],
                   start=(ec == 0), stop=(ec == 1))
            nc.scalar.copy(c0row[j][:], p[0:1, :])
            # a0 = c0 + t1 @ Wq   (col layout) -> a_sb
            p = scrt()
            for hc in range(2):
                mm(p[:, hc:hc + 1], c0row[j][0:1, hc * 128:(hc + 1) * 128], one11[:],
                   start=True, stop=False)
                for ec in range(2):
                    mm(p[:, hc:hc + 1],
                       Wq_sb[:, ec * 256 + hc * 128: ec * 256 + hc * 128 + 128],
                       t1_sb[:, ec:ec + 1], start=False, stop=(ec == 1))
            nc.vector.tensor_copy(a_sb[j][:], p[:, 0:2])
            # M10 = 0
            mm(mbank[j][:, 0:2], zrow[:], iota_col[0:1, 0:2], start=True, stop=False)

        # --- decode loop ---
        for i in range(M):
            for j in range(JPC):
                t = tpool.tile([128, 2 * S], F32, name=f"t{j}", tag=f"t{j}")
                for hc in range(2):
                    nc.scalar.activation(t[:, hc * 256:(hc + 1) * 256],
                                         kT[j][:, hc * 256:(hc + 1) * 256],
                                         AF.Tanh, bias=a_sb[j][:, hc:hc + 1])
                for sc in range(2):
                    for hc in range(2):
                        mm(ubank[j][:, sc:sc + 1],
                           t[:, hc * 256 + sc * 128: hc * 256 + sc * 128 + 128],
                           vptr_col[:, hc:hc + 1],
                           start=(sc == 0 and hc == 0), stop=(hc == 1))
                nc.scalar.activation(tu_tmp[j][:], ubank[j][:, 0:2], AF.Tanh)
                nc.vector.tensor_tensor(tum[j][:, :, 1 + i], tu_tmp[j][:],
                                        mbank[j][:, 0:2], op=OP.add)
                for sc in range(2):
                    nc.tensor.transpose(
                        ubank[j][0:1, 256 + sc * 128: 256 + (sc + 1) * 128],
                        tum[j][:, sc, 1 + i: 2 + i], ident[:])
                nc.vector.tensor_tensor(z_row[j][:], ubank[j][0:1, 256:512],
                                        g10sb[j][0:1, i, :], op=OP.add)
                nc.vector.tensor_reduce(mx[j][:], z_row[j][:],
                                        axis=mybir.AxisListType.X, op=OP.max)
                nc.vector.tensor_scalar(oh_row[j][:], z_row[j][:], mx[j][:],
                                        None, op0=OP.is_equal)
                if i == 0 and j == 0 and DEBUG:
                    zdbg = st([1, S], "zdbg")
                    mdbg = st([1, 1], "mdbg")
                    odbg = st([1, S], "odbg")
                    tdbg = st([1, S], "tdbg")
                    nc.vector.tensor_copy(zdbg[:], z_row[j][:])
                    nc.vector.tensor_copy(mdbg[:], mx[j][:])
                    nc.vector.tensor_copy(odbg[:], oh_row[j][:])
                    nc.vector.tensor_copy(tdbg[:], ubank[j][0:1, 256:512])
                for sc in range(2):
                    mm(abank[j][:, 4 + sc: 5 + sc],
                       oh_row[j][0:1, sc * 128:(sc + 1) * 128], one11[:],
                       start=(sc == 0), stop=True)
                nc.vector.tensor_copy(oh[j][:, :, 1 + i], abank[j][:, 4:6])
                if i == 0:
                    # c1 = c0 + Q[idx0]
                    mm(abank[j][0:1, 16:272], one11[:], c0row[j][:],
                       start=False, stop=False)
                    for sc in range(2):
                        mm(abank[j][0:1, 16:272], oh[j][:, sc, 1:2],
                           Q_sb[j][:, sc * 256:(sc + 1) * 256],
                           start=False, stop=(sc == 1))
                    nc.scalar.copy(c1row[j][:], abank[j][0:1, 16:272])
                if i < M - 1:
                    for sc in range(2):
                        mm(mbank[j][:, sc:sc + 1], cneg[:],
                           oh[j][:, sc, 1 + i: 2 + i], start=False, stop=False)
                    for hc in range(2):
                        mm(abank[j][:, hc:hc + 1],
                           c1row[j][0:1, hc * 128:(hc + 1) * 128], one11[:],
                           start=False, stop=False)
                    for sc in range(2):
                        for hc in range(2):
                            mm(abank[j][:, hc:hc + 1],
                               P_sb[j][:, sc * 256 + hc * 128:
                                        sc * 256 + hc * 128 + 128],
                               oh[j][:, sc, 1 + i: 2 + i],
                               start=False, stop=(sc == 1))
                    nc.vector.tensor_copy(a_sb[j][:], abank[j][:, 0:2])
                    if j == 0 and i <= 2 and DEBUG:
                        adbg = st([128, 2], f"adbg{i}")
                        nc.vector.tensor_copy(adbg[:], abank[j][:, 0:2])

        # --- end phase: logp, reward, actions ---
        lp_sb = st([1, JPC], "lp_sb")
        rw_sb = st([1, JPC], "rw_sb")
        for j in range(JPC):
            tum_cl = st([128, 2, 66], f"tumcl{j}")
            e_t = st([128, 2, 66], f"et{j}")
            prod = st([128, 2, 66], f"prod{j}")
            nc.vector.tensor_scalar(tum_cl[:], tum[j][:], -3.0, None, op0=OP.max)
            nc.scalar.activation(e_t[:], tum_cl[:], AF.Exp, bias=bm10[:],
                                 scale=10.0)
            nc.vector.tensor_tensor(prod[:], tum[j][:], oh[j][:], op=OP.mult)

            sump = ubank[j][0:64, 0:1]
            for sc in range(2):
                mm(sump, e_t[:, sc, 1:65], ones_col[:],
                   start=(sc == 0), stop=(sc == 1))
            lse = st([64, 1], f"lse{j}")
            nc.scalar.activation(lse[:], sump, AF.Ln)

            tselp = ubank[j][0:64, 4:5]
            mm(tselp, negrow[:], one11[:], start=False, stop=False)
            for sc in range(2):
                mm(tselp, prod[:, sc, 1:65], ones_col[:],
                   start=False, stop=(sc == 1))
            tsel_s = st([64, 1], f"tsel{j}")
            nc.vector.tensor_copy(tsel_s[:], tselp)
            logp_col = st([64, 1], f"logp{j}")
            nc.vector.tensor_scalar(logp_col[:], tsel_s[:], 10.0, lse[:],
                                    op0=OP.mult, op1=OP.subtract)
            lpp = ubank[j][0:1, 8:9]
            mm(lpp, logp_col[:], ones_col[0:64, :], start=False, stop=True)
            nc.vector.tensor_copy(lp_sb[0:1, j:j + 1], lpp)

            # reward
            nct = mbank[j][0:64, 16:21]
            nctp = mbank[j][0:64, 24:29]
            for sc in range(2):
                mm(nct, oh[j][:, sc, 1:65], NC_sb[j][:, sc * 5:(sc + 1) * 5],
                   start=(sc == 0), stop=(sc == 1))
            for sc in range(2):
                mm(nctp, oh[j][:, sc, 0:64], NC_sb[j][:, sc * 5:(sc + 1) * 5],
                   start=False, stop=(sc == 1))
            dx = st([64, 1], f"dx{j}")
            dy = st([64, 1], f"dy{j}")
            d2 = st([64, 1], f"d2{j}")
            terms = st([64, 1], f"terms{j}")
            ncp_sb = st([64, 5], f"ncp{j}")
            nc.vector.tensor_copy(ncp_sb[:], nctp)
            nc.vector.tensor_tensor(dx[:], nct[:, 0:1], ncp_sb[:, 2:3],
                                    op=OP.subtract)
            nc.vector.tensor_tensor(dy[:], nct[:, 1:2], ncp_sb[:, 3:4],
                                    op=OP.subtract)
            nc.vector.tensor_tensor(dx[:], dx[:], dx[:], op=OP.mult)
            nc.vector.tensor_tensor(dy[:], dy[:], dy[:], op=OP.mult)
            nc.vector.tensor_tensor(d2[:], dx[:], dy[:], op=OP.add)
            nc.scalar.activation(terms[:], d2[:], AF.Sqrt)
            nc.vector.tensor_tensor(terms[:], terms[:], ncp_sb[:, 4:5], op=OP.add)
            nc.vector.tensor_tensor(terms[:], terms[:], nct[:, 4:5], op=OP.add)
            nc.vector.memset(terms[0:1, :], 0.0)
            rwp = ubank[j][0:1, 12:13]
            mm(rwp, terms[:], ones_col[0:64, :], start=False, stop=True)
            nc.vector.tensor_copy(rw_sb[0:1, j:j + 1], rwp)

            # actions
            idxp = abank[j][0:1, 32:96]
            for sc in range(2):
                mm(idxp, iota_col[:, sc:sc + 1], oh[j][:, sc, 1:65],
                   start=False, stop=(sc == 1))
            act_row = st([1, M], f"actrow{j}")
            nc.vector.tensor_copy(act_row[:], idxp)
            nc.sync.dma_start(act_d.ap()[j].rearrange("(a m) -> a m", a=1), act_row[:])

        nc.sync.dma_start(lp_d.ap().rearrange("(a c) -> a c", a=1), lp_sb[:])
        nc.sync.dma_start(rw_d.ap().rearrange("(a c) -> a c", a=1), rw_sb[:])

    nc.compile()
    return nc


# ----------------------------------------------------------------------------
# host side
# ----------------------------------------------------------------------------

def _gumbel_noise():
    """Gumbel noise exactly as the reference's vmap(scan(categorical)) consumes
    it. Input-independent (fixed seed 42, fixed shapes). Must be extracted
    through the same vmap+scan structure: under the rbg PRNG impl the bit
    stream depends on batching, so per-key eager extraction would differ."""
    import jax
    import jax.numpy as jnp
    from jax import lax
    cpu = jax.devices("cpu")[0]
    with jax.default_device(cpu):
        key = jax.random.key(42)
        bkeys = jax.random.split(key, B)

        def one(bk):
            def step(c, i):
                return c, jax.random.gumbel(jax.random.fold_in(bk, i), (S,),
                                            jnp.float32)
            _, gs = lax.scan(step, 0, jnp.arange(M))
            return gs

        g = np.asarray(jax.jit(jax.vmap(one))(bkeys))
    return g


def _consts():
    ident = np.eye(128, dtype=np.float32)
    cneg = np.zeros((128, 128), np.float32)
    for c in range(32):
        cneg[4 * c:4 * c + 4, 4 * c:4 * c + 4] = -1e8
    iota = np.arange(256, dtype=np.float32).reshape(2, 128).T.copy()
    return ident, cneg, iota


_NC_CACHE = {}


def _get_nc():
    if "nc" not in _NC_CACHE:
        _NC_CACHE["nc"] = build_nc()
    return _NC_CACHE["nc"]


def make_in_maps(cell_embed, original_node, costs, init_w, Whc, bhc, Wv, bv,
                 Wq, Wk, vptr):
    g = (_gumbel_noise() / np.float32(10.0)).astype(np.float32)
    ident, cneg, iota = _consts()
    ncst = np.concatenate(
        [original_node.astype(np.float32), costs.astype(np.float32)[..., None]],
        axis=2)  # [B, S, 5]
    shared = dict(
        Wk=np.ascontiguousarray(Wk, dtype=np.float32),
        Wq=np.ascontiguousarray(Wq, dtype=np.float32),
        Whc=np.ascontiguousarray(Whc, dtype=np.float32),
        Wv=np.ascontiguousarray(Wv, dtype=np.float32),
        bb=np.ascontiguousarray(np.asarray(bhc, np.float32)
                                + np.asarray(bv, np.float32)),
        vptr=np.ascontiguousarray(vptr, dtype=np.float32),
        iw=np.ascontiguousarray(init_w, dtype=np.float32),
        ident=ident, cneg=cneg, iota=iota,
    )
    in_maps = []
    for c in range(NCORES):
        sl = slice(c * JPC, (c + 1) * JPC)
        in_maps.append(dict(
            emb=np.ascontiguousarray(cell_embed[sl], dtype=np.float32),
            ncst=np.ascontiguousarray(ncst[sl], dtype=np.float32),
            g10=np.ascontiguousarray(g[sl], dtype=np.float32),
            **shared,
        ))
    return in_maps


def kernel(cell_embed, original_node, maze, num_cell, costs, init_w, Whc, bhc,
           Wv, bv, Wq, Wk, vptr):
    in_maps = make_in_maps(np.asarray(cell_embed), np.asarray(original_node),
                           np.asarray(costs), np.asarray(init_w),
                           np.asarray(Whc), np.asarray(bhc), np.asarray(Wv),
                           np.asarray(bv), np.asarray(Wq), np.asarray(Wk),
                           np.asarray(vptr))
    nc = _get_nc()
    res = run_bass_kernel_spmd(nc, in_maps, core_ids=list(range(NCORES)))
    lp = np.concatenate([res.results[c]["lp"] for c in range(NCORES)])
    rw = np.concatenate([res.results[c]["rw"] for c in range(NCORES)])
    act = np.concatenate([res.results[c]["act"] for c in range(NCORES)])
    return (lp.astype(np.float32), rw.astype(np.float32),
            np.rint(act).astype(np.int32))


# revision 17
# speedup vs baseline: 1.2911x; 1.2911x over previous
"""Trainium2 Bass kernel for nn_Decoder_86011015069969.

Pointer-network decoder: B=16 items, each runs M=64 sequential steps of
pointer attention over S=256 candidates with Gumbel-max sampling.

Strategy (pure data parallelism): 8 cores x 2 items/core, one SPMD program.
Host precomputes only input-independent PRNG constants (Gumbel noise from
fixed seed 42 — exactly what jax.random.categorical adds to the logits).

Device-side algebra (validated against the jax reference on host):
  - query recurrence folded:  a_i = q_i @ Wq = c1 + P[idx_{i-1}],
    P = emb @ Wv[256:] @ Wq,  Q = emb @ Wv[:256] @ Wq,  c1 = c0 + Q[idx_0]
  - sampling: argmax_j( tanh(u_j) + M10_j + g_ij/10 ), scale-invariant vs
    reference argmax(10*tanh(u) + mask*(-1e9) + g); M10 in {0, -1e8}
  - log_softmax via fixed-shift logsumexp: lse = 10 + log(sum exp(lg-10))
"""
import numpy as np
from contextlib import ExitStack

import concourse.bass as bass
import concourse.bacc as bacc
import concourse.tile as tile
import concourse.mybir as mybir
from concourse.bass_utils import run_bass_kernel_spmd

F32 = mybir.dt.float32
AF = mybir.ActivationFunctionType
OP = mybir.AluOpType

B, S, E, H, M = 16, 256, 256, 256, 64
NCORES = 8
JPC = B // NCORES  # items per core = 2
DEBUG = False


# ----------------------------------------------------------------------------
# device program
# ----------------------------------------------------------------------------

def build_nc():
    nc = bacc.Bacc("TRN2", target_bir_lowering=False, debug=False,
                   enable_asserts=True)

    # --- DRAM I/O (per core) ---
    emb_d = nc.dram_tensor("emb", [JPC, S, E], F32, kind="ExternalInput")
    ncst_d = nc.dram_tensor("ncst", [JPC, S, 5], F32, kind="ExternalInput")
    g10c_d = nc.dram_tensor("g10c", [JPC, 128, 2, M], F32, kind="ExternalInput")
    Wk_d = nc.dram_tensor("Wk", [E, H], F32, kind="ExternalInput")
    Wq_d = nc.dram_tensor("Wq", [E, H], F32, kind="ExternalInput")
    Whc_d = nc.dram_tensor("Whc", [E, E], F32, kind="ExternalInput")
    Wv_d = nc.dram_tensor("Wv", [2 * E, E], F32, kind="ExternalInput")
    bb_d = nc.dram_tensor("bb", [E], F32, kind="ExternalInput")      # bhc + bv
    vptr_d = nc.dram_tensor("vptr", [H], F32, kind="ExternalInput")
    iw_d = nc.dram_tensor("iw", [2 * E], F32, kind="ExternalInput")
    ident_d = nc.dram_tensor("ident", [128, 128], F32, kind="ExternalInput")
    cneg_d = nc.dram_tensor("cneg", [128, 128], F32, kind="ExternalInput")
    iota_d = nc.dram_tensor("iota", [128, 2], F32, kind="ExternalInput")

    lp_d = nc.dram_tensor("lp", [JPC], F32, kind="ExternalOutput")
    rw_d = nc.dram_tensor("rw", [JPC], F32, kind="ExternalOutput")
    act_d = nc.dram_tensor("act", [JPC, M], F32, kind="ExternalOutput")

    with tile.TileContext(nc) as tc, ExitStack() as ctx:
        sb = ctx.enter_context(tc.tile_pool(name="sb", bufs=1))
        tpool = ctx.enter_context(tc.tile_pool(name="tp", bufs=2))
        ps = ctx.enter_context(tc.tile_pool(name="ps", bufs=1, space="PSUM"))
        scr = ctx.enter_context(tc.tile_pool(name="scr", bufs=2, space="PSUM"))

        def mm(out, lhsT, rhs, start, stop):
            nc.tensor.matmul(out, lhsT, rhs, start=start, stop=stop,
                             skip_group_check=True)

        def st(shape, nm):
            return sb.tile(shape, F32, name=nm, tag=nm)

        def scrt():
            return scr.tile([128, 256], F32, name="scrA", tag="scrA")

        # --- SBUF constants / weights ---
        Wk_sb = st([128, 2 * H], "Wk_sb")
        Wq_sb = st([128, 2 * H], "Wq_sb")
        Whc_sb = st([128, 2 * E], "Whc_sb")
        Wv_sb = st([128, 4 * E], "Wv_sb")
        bb_row = st([1, E], "bb_row")
        vptr_col = st([128, 2], "vptr_col")
        iw_col = st([128, 4], "iw_col")
        ident = st([128, 128], "ident_sb")
        cneg = st([128, 128], "cneg_sb")
        iota_col = st([128, 2], "iota_sb")
        nc.sync.dma_start(Wk_sb[:].rearrange("p (r c) -> p r c", r=2),
                          Wk_d.ap().rearrange("(r p) c -> p r c", p=128))
        nc.sync.dma_start(Wq_sb[:].rearrange("p (r c) -> p r c", r=2),
                          Wq_d.ap().rearrange("(r p) c -> p r c", p=128))
        nc.sync.dma_start(Whc_sb[:].rearrange("p (r c) -> p r c", r=2),
                          Whc_d.ap().rearrange("(r p) c -> p r c", p=128))
        nc.sync.dma_start(Wv_sb[:].rearrange("p (r c) -> p r c", r=4),
                          Wv_d.ap().rearrange("(r p) c -> p r c", p=128))
        nc.sync.dma_start(bb_row[:], bb_d.ap().rearrange("(a c) -> a c", a=1))
        nc.sync.dma_start(vptr_col[:], vptr_d.ap().rearrange("(c p) -> p c", p=128))
        nc.sync.dma_start(iw_col[:], iw_d.ap().rearrange("(c p) -> p c", p=128))
        nc.sync.dma_start(ident[:], ident_d.ap())
        nc.sync.dma_start(cneg[:], cneg_d.ap())
        nc.sync.dma_start(iota_col[:], iota_d.ap())

        one11 = st([1, 1], "one11")
        ones_col = st([128, 1], "ones_col")
        inv_col = st([128, 1], "inv_col")
        zrow = st([1, 128], "zrow")
        onesrow = st([1, 128], "onesrow")
        negrow = st([1, M], "negrow")
        bm10 = st([128, 1], "bm10")
        nc.vector.memset(one11[:], 1.0)
        nc.vector.memset(ones_col[:], 1.0)
        nc.vector.memset(inv_col[:], 1.0 / S)
        nc.vector.memset(zrow[:], 0.0)
        nc.vector.memset(onesrow[:], 1.0)
        nc.vector.memset(negrow[:], -1.0)
        nc.vector.memset(bm10[:], -10.0)

        # --- shared precompute: WvaT/WvbT -> Gq (= Wva@Wq), G (= Wvb@Wq) ---
        WvaT = st([128, 2 * E], "WvaT")
        WvbT = st([128, 2 * E], "WvbT")
        for half, dst in ((0, WvaT), (1, WvbT)):
            for cc in range(2):
                p = scrt()
                for rc in range(2):
                    nc.tensor.transpose(
                        p[:, rc * 128:(rc + 1) * 128],
                        Wv_sb[:, (2 * half + rc) * 256 + cc * 128:
                              (2 * half + rc) * 256 + (cc + 1) * 128],
                        ident[:])
                nc.scalar.copy(dst[:, cc * 256:(cc + 1) * 256], p[:])
        G_sb = st([128, 2 * H], "G_sb")
        Gq_sb = st([128, 2 * H], "Gq_sb")
        for src, dst in ((WvbT, G_sb), (WvaT, Gq_sb)):
            for vc in range(2):
                p = scrt()
                for mc in range(2):
                    mm(p[:], src[:, mc * 256 + vc * 128: mc * 256 + vc * 128 + 128],
                       Wq_sb[:, mc * 256:(mc + 1) * 256],
                       start=(mc == 0), stop=(mc == 1))
                nc.scalar.copy(dst[:, vc * 256:(vc + 1) * 256], p[:])

        # t1 = init_w @ Wv  (shared, col layout)
        t1_sb = st([128, 2], "t1_sb")
        p = scrt()
        for ec in range(2):
            for rc in range(4):
                mm(p[:, ec:ec + 1],
                   Wv_sb[:, rc * 256 + ec * 128: rc * 256 + ec * 128 + 128],
                   iw_col[:, rc:rc + 1], start=(rc == 0), stop=(rc == 3))
        nc.vector.tensor_copy(t1_sb[:], p[:, 0:2])

        # --- per-item tiles ---
        embN, embT, kT, P_sb, Q_sb, NC_sb = [], [], [], [], [], []
        g10c, zcol, gmcol, pmax = [], [], [], []
        tum, oh, a_sb, c0row, c1row, tu_tmp = [], [], [], [], [], []
        z_row, oh_row, mx, meanv, hv_sb = [], [], [], [], []
        for j in range(JPC):
            embN.append(st([128, 2 * E], f"embN{j}"))
            embT.append(st([128, 2 * S], f"embT{j}"))
            kT.append(st([128, 2 * S], f"kT{j}"))
            P_sb.append(st([128, 2 * H], f"P{j}"))
            Q_sb.append(st([128, 2 * H], f"Q{j}"))
            NC_sb.append(st([128, 10], f"NC{j}"))
            g10c.append(st([128, 2, M], f"g10c{j}"))
            zcol.append(st([128, 2], f"zcol{j}"))
            gmcol.append(st([128, 2], f"gmcol{j}"))
            pmax.append(st([128, 1], f"pmax{j}"))
            tum.append(st([128, 2, 66], f"tum{j}"))
            oh.append(st([128, 2, 66], f"oh{j}"))
            a_sb.append(st([128, 2], f"a_sb{j}"))
            c0row.append(st([1, H], f"c0row{j}"))
            c1row.append(st([1, H], f"c1row{j}"))
            tu_tmp.append(st([128, 2], f"tu_tmp{j}"))
            z_row.append(st([1, S], f"z_row{j}"))
            oh_row.append(st([1, S], f"oh_row{j}"))
            mx.append(st([1, 1], f"mx{j}"))
            meanv.append(st([128, 2], f"meanv{j}"))
            hv_sb.append(st([128, 2], f"hv{j}"))

        ubank = [ps.tile([128, 512], F32, name=f"ub{j}", tag=f"ub{j}")
                 for j in range(JPC)]
        abank = [ps.tile([128, 512], F32, name=f"ab{j}", tag=f"ab{j}")
                 for j in range(JPC)]
        mbank = [ps.tile([128, 512], F32, name=f"mb{j}", tag=f"mb{j}")
                 for j in range(JPC)]

        for j in range(JPC):
            nc.sync.dma_start(embN[j][:].rearrange("p (r c) -> p r c", r=2),
                              emb_d.ap()[j].rearrange("(r p) c -> p r c", p=128))
            nc.sync.dma_start(NC_sb[j][:].rearrange("p (r c) -> p r c", r=2),
                              ncst_d.ap()[j].rearrange("(r p) c -> p r c", p=128))
            nc.sync.dma_start(g10c[j][:], g10c_d.ap()[j])
            nc.vector.memset(tum[j][:], 0.0)
            nc.vector.memset(oh[j][:], 0.0)

        # --- per-item setup ---
        for j in range(JPC):
            # embT via PE transposes
            for ec in range(2):
                p = scrt()
                for sc in range(2):
                    nc.tensor.transpose(
                        p[:, sc * 128:(sc + 1) * 128],
                        embN[j][:, sc * 256 + ec * 128: sc * 256 + ec * 128 + 128],
                        ident[:])
                nc.scalar.copy(embT[j][:, ec * 256:(ec + 1) * 256], p[:])
            # kT[h,s] = sum_e Wk[e,h] * embT[e,s]
            for hc in range(2):
                p = scrt()
                for ec in range(2):
                    mm(p[:], Wk_sb[:, ec * 256 + hc * 128: ec * 256 + hc * 128 + 128],
                       embT[j][:, ec * 256:(ec + 1) * 256],
                       start=(ec == 0), stop=(ec == 1))
                nc.scalar.copy(kT[j][:, hc * 256:(hc + 1) * 256], p[:])
            # P = emb @ G ; Q = emb @ Gq   (natural [s, h] layout)
            for gsb, dst in ((G_sb, P_sb[j]), (Gq_sb, Q_sb[j])):
                for sc in range(2):
                    p = scrt()
                    for ec in range(2):
                        mm(p[:],
                           embT[j][:, ec * 256 + sc * 128: ec * 256 + sc * 128 + 128],
                           gsb[:, ec * 256:(ec + 1) * 256],
                           start=(ec == 0), stop=(ec == 1))
                    nc.scalar.copy(dst[:, sc * 256:(sc + 1) * 256], p[:])
            # meanv_col = mean_s emb
            p = scrt()
            for ec in range(2):
                for sc in range(2):
                    mm(p[:, ec:ec + 1],
                       embN[j][:, sc * 256 + ec * 128: sc * 256 + ec * 128 + 128],
                       inv_col[:], start=(sc == 0), stop=(sc == 1))
            nc.vector.tensor_copy(meanv[j][:], p[:, 0:2])
            # hv = meanv @ Whc + (bhc + bv)   (col layout)
            p = scrt()
            for oc in range(2):
                mm(p[:, oc:oc + 1], bb_row[0:1, oc * 128:(oc + 1) * 128], one11[:],
                   start=True, stop=False)
                for ec in range(2):
                    mm(p[:, oc:oc + 1],
                       Whc_sb[:, ec * 256 + oc * 128: ec * 256 + oc * 128 + 128],
                       meanv[j][:, ec:ec + 1], start=False, stop=(ec == 1))
            nc.vector.tensor_copy(hv_sb[j][:], p[:, 0:2])
            # c0_row = hv @ Wq
            p = scrt()
            for ec in range(2):
                mm(p[0:1, :], hv_sb[j][:, ec:ec + 1],
                   Wq_sb[:, ec * 256:(ec + 1) * 256],
                   start=(ec == 0), stop=(ec == 1))
            nc.scalar.copy(c0row[j][:], p[0:1, :])
            # a0 = c0 + t1 @ Wq   (col layout) -> a_sb
            p = scrt()
            for hc in range(2):
                mm(p[:, hc:hc + 1], c0row[j][0:1, hc * 128:(hc + 1) * 128], one11[:],
                   start=True, stop=False)
                for ec in range(2):
                    mm(p[:, hc:hc + 1],
                       Wq_sb[:, ec * 256 + hc * 128: ec * 256 + hc * 128 + 128],
                       t1_sb[:, ec:ec + 1], start=False, stop=(ec == 1))
            nc.vector.tensor_copy(a_sb[j][:], p[:, 0:2])
            # M10 = 0
            mm(mbank[j][:, 0:2], zrow[:], iota_col[0:1, 0:2], start=True, stop=False)

        # --- decode loop ---
        for i in range(M):
            for j in range(JPC):
                t = tpool.tile([128, 2 * S], F32, name=f"t{j}", tag=f"t{j}")
                nc.vector.tensor_tensor(gmcol[j][:], g10c[j][:, :, i],
                                        mbank[j][:, 0:2], op=OP.add)
                for hc in range(2):
                    nc.scalar.activation(t[:, hc * 256:(hc + 1) * 256],
                                         kT[j][:, hc * 256:(hc + 1) * 256],
                                         AF.Tanh, bias=a_sb[j][:, hc:hc + 1])
                for sc in range(2):
                    for hc in range(2):
                        mm(ubank[j][:, sc:sc + 1],
                           t[:, hc * 256 + sc * 128: hc * 256 + sc * 128 + 128],
                           vptr_col[:, hc:hc + 1],
                           start=(sc == 0 and hc == 0), stop=(hc == 1))
                nc.scalar.activation(tu_tmp[j][:], ubank[j][:, 0:2], AF.Tanh)
                nc.vector.tensor_tensor(zcol[j][:], tu_tmp[j][:],
                                        gmcol[j][:], op=OP.add)
                nc.vector.tensor_reduce(pmax[j][:], zcol[j][:],
                                        axis=mybir.AxisListType.X, op=OP.max)
                nc.tensor.transpose(ubank[j][0:1, 256:384], pmax[j][:],
                                    ident[:])
                nc.vector.tensor_reduce(mx[j][:], ubank[j][0:1, 256:384],
                                        axis=mybir.AxisListType.X, op=OP.max)
                mm(ubank[j][:, 4:5], onesrow[:], mx[j][:],
                   start=False, stop=False)
                nc.vector.tensor_scalar(oh[j][:, :, 1 + i], zcol[j][:],
                                        ubank[j][:, 4:5], None, op0=OP.is_equal)
                nc.vector.tensor_tensor(tum[j][:, :, 1 + i], tu_tmp[j][:],
                                        mbank[j][:, 0:2], op=OP.add)
                if i == 0 and j == 0 and DEBUG:
                    zdbg = st([1, S], "zdbg")
                    mdbg = st([1, 1], "mdbg")
                    odbg = st([1, S], "odbg")
                    tdbg = st([1, S], "tdbg")
                    nc.vector.tensor_copy(zdbg[:], z_row[j][:])
                    nc.vector.tensor_copy(mdbg[:], mx[j][:])
                    nc.vector.tensor_copy(odbg[:], oh_row[j][:])
                    nc.vector.tensor_copy(tdbg[:], ubank[j][0:1, 256:512])
                if i == 0:
                    # c1 = c0 + Q[idx0]
                    mm(abank[j][0:1, 16:272], one11[:], c0row[j][:],
                       start=True, stop=False)
                    for sc in range(2):
                        mm(abank[j][0:1, 16:272], oh[j][:, sc, 1:2],
                           Q_sb[j][:, sc * 256:(sc + 1) * 256],
                           start=False, stop=(sc == 1))
                    nc.scalar.copy(c1row[j][:], abank[j][0:1, 16:272])
                if i < M - 1:
                    for hc in range(2):
                        mm(abank[j][:, hc:hc + 1],
                           c1row[j][0:1, hc * 128:(hc + 1) * 128], one11[:],
                           start=(hc == 0), stop=False)
                    for sc in range(2):
                        for hc in range(2):
                            mm(abank[j][:, hc:hc + 1],
                               P_sb[j][:, sc * 256 + hc * 128:
                                        sc * 256 + hc * 128 + 128],
                               oh[j][:, sc, 1 + i: 2 + i],
                               start=False, stop=(sc == 1))
                    nc.vector.tensor_copy(a_sb[j][:], abank[j][:, 0:2])
                    for sc in range(2):
                        mm(mbank[j][:, sc:sc + 1], cneg[:],
                           oh[j][:, sc, 1 + i: 2 + i], start=False, stop=False)
                    if j == 0 and i <= 2 and DEBUG:
                        adbg = st([128, 2], f"adbg{i}")
                        nc.vector.tensor_copy(adbg[:], abank[j][:, 0:2])

        # --- end phase: logp, reward, actions ---
        lp_sb = st([1, JPC], "lp_sb")
        rw_sb = st([1, JPC], "rw_sb")
        for j in range(JPC):
            tum_cl = st([128, 2, 66], f"tumcl{j}")
            e_t = st([128, 2, 66], f"et{j}")
            prod = st([128, 2, 66], f"prod{j}")
            nc.vector.tensor_scalar(tum_cl[:], tum[j][:], -3.0, None, op0=OP.max)
            nc.scalar.activation(e_t[:], tum_cl[:], AF.Exp, bias=bm10[:],
                                 scale=10.0)
            nc.vector.tensor_tensor(prod[:], tum[j][:], oh[j][:], op=OP.mult)

            sump = ubank[j][0:64, 0:1]
            for sc in range(2):
                mm(sump, e_t[:, sc, 1:65], ones_col[:],
                   start=(sc == 0), stop=(sc == 1))
            lse = st([64, 1], f"lse{j}")
            nc.scalar.activation(lse[:], sump, AF.Ln)

            tselp = ubank[j][0:64, 4:5]
            mm(tselp, negrow[:], one11[:], start=False, stop=False)
            for sc in range(2):
                mm(tselp, prod[:, sc, 1:65], ones_col[:],
                   start=False, stop=(sc == 1))
            tsel_s = st([64, 1], f"tsel{j}")
            nc.vector.tensor_copy(tsel_s[:], tselp)
            logp_col = st([64, 1], f"logp{j}")
            nc.vector.tensor_scalar(logp_col[:], tsel_s[:], 10.0, lse[:],
                                    op0=OP.mult, op1=OP.subtract)
            lpp = ubank[j][0:1, 8:9]
            mm(lpp, logp_col[:], ones_col[0:64, :], start=False, stop=True)
            nc.vector.tensor_copy(lp_sb[0:1, j:j + 1], lpp)

            # reward
            nct = mbank[j][0:64, 16:21]
            nctp = mbank[j][0:64, 24:29]
            for sc in range(2):
                mm(nct, oh[j][:, sc, 1:65], NC_sb[j][:, sc * 5:(sc + 1) * 5],
                   start=(sc == 0), stop=(sc == 1))
            for sc in range(2):
                mm(nctp, oh[j][:, sc, 0:64], NC_sb[j][:, sc * 5:(sc + 1) * 5],
                   start=False, stop=(sc == 1))
            dx = st([64, 1], f"dx{j}")
            dy = st([64, 1], f"dy{j}")
            d2 = st([64, 1], f"d2{j}")
            terms = st([64, 1], f"terms{j}")
            ncp_sb = st([64, 5], f"ncp{j}")
            nc.vector.tensor_copy(ncp_sb[:], nctp)
            nc.vector.tensor_tensor(dx[:], nct[:, 0:1], ncp_sb[:, 2:3],
                                    op=OP.subtract)
            nc.vector.tensor_tensor(dy[:], nct[:, 1:2], ncp_sb[:, 3:4],
                                    op=OP.subtract)
            nc.vector.tensor_tensor(dx[:], dx[:], dx[:], op=OP.mult)
            nc.vector.tensor_tensor(dy[:], dy[:], dy[:], op=OP.mult)
            nc.vector.tensor_tensor(d2[:], dx[:], dy[:], op=OP.add)
            nc.scalar.activation(terms[:], d2[:], AF.Sqrt)
            nc.vector.tensor_tensor(terms[:], terms[:], ncp_sb[:, 4:5], op=OP.add)
            nc.vector.tensor_tensor(terms[:], terms[:], nct[:, 4:5], op=OP.add)
            nc.vector.memset(terms[0:1, :], 0.0)
            rwp = ubank[j][0:1, 12:13]
            mm(rwp, terms[:], ones_col[0:64, :], start=False, stop=True)
            nc.vector.tensor_copy(rw_sb[0:1, j:j + 1], rwp)

            # actions
            idxp = abank[j][0:1, 32:96]
            for sc in range(2):
                mm(idxp, iota_col[:, sc:sc + 1], oh[j][:, sc, 1:65],
                   start=False, stop=(sc == 1))
            act_row = st([1, M], f"actrow{j}")
            nc.vector.tensor_copy(act_row[:], idxp)
            nc.sync.dma_start(act_d.ap()[j].rearrange("(a m) -> a m", a=1), act_row[:])

        nc.sync.dma_start(lp_d.ap().rearrange("(a c) -> a c", a=1), lp_sb[:])
        nc.sync.dma_start(rw_d.ap().rearrange("(a c) -> a c", a=1), rw_sb[:])

    nc.compile()
    return nc


# ----------------------------------------------------------------------------
# host side
# ----------------------------------------------------------------------------

def _gumbel_noise():
    """Gumbel noise exactly as the reference's vmap(scan(categorical)) consumes
    it. Input-independent (fixed seed 42, fixed shapes). Must be extracted
    through the same vmap+scan structure: under the rbg PRNG impl the bit
    stream depends on batching, so per-key eager extraction would differ."""
    import jax
    import jax.numpy as jnp
    from jax import lax
    cpu = jax.devices("cpu")[0]
    with jax.default_device(cpu):
        key = jax.random.key(42)
        bkeys = jax.random.split(key, B)

        def one(bk):
            def step(c, i):
                return c, jax.random.gumbel(jax.random.fold_in(bk, i), (S,),
                                            jnp.float32)
            _, gs = lax.scan(step, 0, jnp.arange(M))
            return gs

        g = np.asarray(jax.jit(jax.vmap(one))(bkeys))
    return g


def _consts():
    ident = np.eye(128, dtype=np.float32)
    cneg = np.zeros((128, 128), np.float32)
    for c in range(32):
        cneg[4 * c:4 * c + 4, 4 * c:4 * c + 4] = -1e8
    iota = np.arange(256, dtype=np.float32).reshape(2, 128).T.copy()
    return ident, cneg, iota


_NC_CACHE = {}


def _get_nc():
    if "nc" not in _NC_CACHE:
        _NC_CACHE["nc"] = build_nc()
    return _NC_CACHE["nc"]


def make_in_maps(cell_embed, original_node, costs, init_w, Whc, bhc, Wv, bv,
                 Wq, Wk, vptr):
    g = (_gumbel_noise() / np.float32(10.0)).astype(np.float32)
    ident, cneg, iota = _consts()
    ncst = np.concatenate(
        [original_node.astype(np.float32), costs.astype(np.float32)[..., None]],
        axis=2)  # [B, S, 5]
    shared = dict(
        Wk=np.ascontiguousarray(Wk, dtype=np.float32),
        Wq=np.ascontiguousarray(Wq, dtype=np.float32),
        Whc=np.ascontiguousarray(Whc, dtype=np.float32),
        Wv=np.ascontiguousarray(Wv, dtype=np.float32),
        bb=np.ascontiguousarray(np.asarray(bhc, np.float32)
                                + np.asarray(bv, np.float32)),
        vptr=np.ascontiguousarray(vptr, dtype=np.float32),
        iw=np.ascontiguousarray(init_w, dtype=np.float32),
        ident=ident, cneg=cneg, iota=iota,
    )
    # column-layout gumbel: g10c[b, p, c, i] = g[b, i, c*128 + p]
    gc = np.ascontiguousarray(
        g.reshape(B, M, 2, 128).transpose(0, 3, 2, 1), dtype=np.float32)
    in_maps = []
    for c in range(NCORES):
        sl = slice(c * JPC, (c + 1) * JPC)
        in_maps.append(dict(
            emb=np.ascontiguousarray(cell_embed[sl], dtype=np.float32),
            ncst=np.ascontiguousarray(ncst[sl], dtype=np.float32),
            g10=np.ascontiguousarray(g[sl], dtype=np.float32),
            g10c=np.ascontiguousarray(gc[sl], dtype=np.float32),
            **shared,
        ))
    return in_maps


def kernel(cell_embed, original_node, maze, num_cell, costs, init_w, Whc, bhc,
           Wv, bv, Wq, Wk, vptr):
    in_maps = make_in_maps(np.asarray(cell_embed), np.asarray(original_node),
                           np.asarray(costs), np.asarray(init_w),
                           np.asarray(Whc), np.asarray(bhc), np.asarray(Wv),
                           np.asarray(bv), np.asarray(Wq), np.asarray(Wk),
                           np.asarray(vptr))
    nc = _get_nc()
    res = run_bass_kernel_spmd(nc, in_maps, core_ids=list(range(NCORES)))
    lp = np.concatenate([res.results[c]["lp"] for c in range(NCORES)])
    rw = np.concatenate([res.results[c]["rw"] for c in range(NCORES)])
    act = np.concatenate([res.results[c]["act"] for c in range(NCORES)])
    return (lp.astype(np.float32), rw.astype(np.float32),
            np.rint(act).astype(np.int32))
